# revision 34
# baseline (speedup 1.0000x reference)
"""Multi-head causal self-attention (B=1, S=4096, D=1024, H=16) on 8 TRN2
NeuronCores, tensor-parallel over heads (2 heads per core).

Engine plan (per core, modeled):
  - PE: QKV projection bf16 (Q^T/K^T in weight-stationary form, V in
    X-stationary form so V lands as [s, f] with no transposes), scores via
    fp8e4 DoubleRow (0.5 cyc/row), causal mask added in-PSUM by a ramp
    matmul (-1e30*max(t-s,0) = A^T B), AV with P^T stationary and [V|1]
    moving (N=65 per pass, denominator rides along as column 64), output
    projection bf16.
  - ACT: only the softmax exp (scale=1/8 folded in), one inst per t-tile.
  - DVE: psum->sbuf copies for Q/K (bias add + fp8 cast), V copies,
    reciprocal + per-partition divide (the [s, f] layout makes the
    denominator a per-partition scalar).
  - Pool: output-projection psum->sbuf copies.
  - SP: all DMAs, including the DMA-XBAR transpose that turns ot [s,f]
    into ot^T [f,s] for the output projection.

Dependency hygiene: the tile framework tracks deps at tile granularity, so
qt/kt/v live in per-chunk tiles, scores in per-t-tile pool tiles, and the
QKV work for chunk j+2 is emitted in pieces interleaved between the t-tiles
of chunk j so the PE always has independent work while ACT runs exp.

Host folds bv through Wo into the final bias (exact), sums the 8 bf16
partials in fp32, and adds bo.
"""

import sys

sys.path.insert(0, "/opt/trn_rl_repo")

import functools
import numpy as np
import ml_dtypes

D = 1024
H = 16
HD = 64
NCORES = 8
HPC = H // NCORES  # 2 heads per core
P = 128
CH = 512
S = 4096
NCHUNK = S // CH  # 8
ND = D // P  # 8
NT = S // P  # 32

USE_FP8 = True  # fp8e4 DoubleRow for the QK^T matmul

BF16 = ml_dtypes.bfloat16

# emission-order labels per (engine, opcode) for profiling
LABELS = {}


def _lab(key, s):
    LABELS.setdefault(key, []).append(s)


def build_nc():
    import concourse.bacc as bacc
    import concourse.mybir as mybir
    from concourse import tile

    f32 = mybir.dt.float32
    bf16 = mybir.dt.bfloat16
    f8 = mybir.dt.float8e4
    ADD = mybir.AluOpType.add
    MULT = mybir.AluOpType.mult
    EXP = mybir.ActivationFunctionType.Exp
    DR = mybir.MatmulPerfMode.DoubleRow

    nc = bacc.Bacc("TRN2", target_bir_lowering=False, debug=False)

    xt_d = nc.dram_tensor("xt", [D, S], bf16, kind="ExternalInput")
    wqk_d = nc.dram_tensor("wqk", [D, 2 * P], bf16, kind="ExternalInput")
    wqk0_d = nc.dram_tensor("wqk0", [D, 2 * P], bf16, kind="ExternalInput")
    bqk0_d = nc.dram_tensor("bqk0", [2 * P], f32, kind="ExternalInput")
    wv_d = nc.dram_tensor("wv", [D, P], bf16, kind="ExternalInput")
    wo_d = nc.dram_tensor("wo", [P, D], bf16, kind="ExternalInput")
    bqk_d = nc.dram_tensor("bqk", [2 * P], f32, kind="ExternalInput")
    maskA_d = nc.dram_tensor("maskA", [P, P], bf16, kind="ExternalInput")
    maskB_d = nc.dram_tensor("maskB", [P, P], bf16, kind="ExternalInput")
    ident_d = nc.dram_tensor("ident", [P, P], f32, kind="ExternalInput")
    yt_d = nc.dram_tensor("yt", [D, S], bf16, kind="ExternalOutput")

    with tile.TileContext(nc) as tc:
        with (
            tc.tile_pool(name="consts", bufs=1) as consts,
            tc.tile_pool(name="xtp", bufs=6) as xtp,
            tc.tile_pool(name="f8p", bufs=4) as f8p,
            tc.tile_pool(name="ptp", bufs=36) as ptp,
            tc.tile_pool(name="otp", bufs=6) as otp,
            tc.tile_pool(name="otTp", bufs=2) as otTp,
            tc.tile_pool(name="rcp", bufs=4) as rcp,
            tc.tile_pool(name="ytp", bufs=2) as ytp,
            tc.tile_pool(name="scp", bufs=2, space="PSUM") as scp,
            tc.tile_pool(name="avp", bufs=1, space="PSUM") as avp,
            tc.tile_pool(name="qkvp", bufs=1, space="PSUM") as qkvp,
            tc.tile_pool(name="ytpp", bufs=1, space="PSUM") as ytpp,
        ):
            # ---- persistent SBUF ----
            wqk_sb = consts.tile([P, ND, 2 * P], bf16)
            wv_sb = consts.tile([P, ND, P], bf16)
            wo_sb = consts.tile([P, ND, P], bf16)
            bqk_sb = consts.tile([P, 2], f32)
            wqk0_sb = consts.tile([P, ND, 2 * P], bf16)
            bqk0_sb = consts.tile([P, 2], f32)
            maskA_sb = consts.tile([P, P], bf16)
            maskB_sb = consts.tile([P, P], bf16)
            ident_sb = consts.tile([P, P], f32)
            # per-chunk tensors (separate tiles -> precise deps)
            v_sbs = [
                consts.tile([P, 4, HPC, 65], bf16, name=f"v{c}")
                for c in range(NCHUNK)
            ]
            if USE_FP8:
                qt_drs = [
                    consts.tile([64, 2, CH], f8, name=f"qdr{c}")
                    for c in range(NCHUNK)
                ]
                kt_drs = [
                    consts.tile([64, 2, CH], f8, name=f"kdr{c}")
                    for c in range(NCHUNK)
                ]
                # chunks 0-1 also get bf16 Q/K so the early scores skip
                # the fp8 rearrange round-trip (startup critical path)
                qt_sbs = [
                    consts.tile([P, CH], bf16, name=f"q{c}") for c in range(3)
                ]
                kt_sbs = [
                    consts.tile([P, CH], bf16, name=f"k{c}") for c in range(3)
                ]
            else:
                qt_sbs = [
                    consts.tile([P, CH], bf16, name=f"q{c}")
                    for c in range(NCHUNK)
                ]
                kt_sbs = [
                    consts.tile([P, CH], bf16, name=f"k{c}")
                    for c in range(NCHUNK)
                ]

            w_first_sb = wqk0_sb if USE_FP8 else wqk_sb
            w_first_d = wqk0_d if USE_FP8 else wqk_d
            b_first_sb = bqk0_sb if USE_FP8 else bqk_sb
            b_first_d = bqk0_d if USE_FP8 else bqk_d
            nc.sync.dma_start(
                w_first_sb[:, :, 0:P],
                w_first_d[:, 0:P].rearrange("(dt p) f -> p dt f", p=P),
            )
            for c in range(NCHUNK):
                nc.gpsimd.memset(v_sbs[c][:, :, :, 64:65], 1.0)

            # AV accumulators: [s, head, stile pair, 0:65 used]
            # two 1-bank tiles so early stile divs don't serialize later AVs
            avs = [
                avp.tile([P, HPC, 2, P], f32, name=f"av{b}") for b in range(2)
            ]

            def make_qkv_pieces(c):
                """QKV for chunk c as a list of emit-closures.

                X^T arrives in two half-tiles so the first Q matmuls can
                start as soon as the first half lands.
                """
                xt_h = [
                    xtp.tile([P, ND // 2, CH], bf16, tag="xt", name="xt")
                    for _ in range(2)
                ]
                for g in range(2):
                    nc.sync.dma_start(
                        xt_h[g][:],
                        xt_d[
                            g * (D // 2) : (g + 1) * (D // 2),
                            c * CH : (c + 1) * CH,
                        ].rearrange("(dt p) s -> p dt s", p=P),
                    )

                def xt_t(d):
                    return xt_h[d // 4][:, d % 4]

                def qk_piece(t):  # 0 = Q, 1 = K
                    def f():
                        natural = USE_FP8 and c <= 1
                        w_sb = wqk0_sb if natural else wqk_sb
                        b_sb = bqk0_sb if natural else bqk_sb
                        pool = ytpp if (c == 0 and t == 1) else qkvp
                        tag = "ytps" if (c == 0 and t == 1) else "qv"
                        ps = pool.tile([P, CH], f32, tag=tag, name=f"qk{t}")
                        for d in range(ND):
                            _lab("mm", f"qk{t}.c{c}.d{d}")
                            nc.tensor.matmul(
                                ps[:],
                                w_sb[:, d, t * P : (t + 1) * P],
                                xt_t(d)[:, :],
                                start=(d == 0),
                                stop=(d == ND - 1),
                            )
                        if USE_FP8:
                            if natural:
                                nc.vector.tensor_scalar(
                                    out=[qt_sbs, kt_sbs][t][c][:],
                                    in0=ps[:],
                                    scalar1=b_sb[:, t : t + 1],
                                    scalar2=None,
                                    op0=ADD,
                                )
                            st = f8p.tile([P, CH], f8, tag="f8", name="f8st")
                            nc.vector.tensor_scalar(
                                out=st[:],
                                in0=ps[:],
                                scalar1=b_sb[:, t : t + 1],
                                scalar2=None,
                                op0=ADD,
                            )
                            dr_t = [qt_drs, kt_drs][t][c]
                            if natural:
                                # natural [h0|h1] order -> dr layout via 4
                                # transfers (off the startup critical path)
                                for h in range(HPC):
                                    for i in range(2):
                                        nc.sync.dma_start(
                                            dr_t[32 * h : 32 * h + 32, i, :],
                                            st[
                                                64 * h + 32 * i : 64 * h
                                                + 32 * i
                                                + 32,
                                                :,
                                            ],
                                        )
                            else:
                                nc.sync.dma_start(
                                    dr_t[0:64, 0, :], st[0:64, :]
                                )
                                nc.sync.dma_start(
                                    dr_t[0:64, 1, :], st[64:128, :]
                                )
                        else:
                            dest = [qt_sbs, kt_sbs][t][c]
                            nc.vector.tensor_scalar(
                                out=dest[:],
                                in0=ps[:],
                                scalar1=bqk_sb[:, t : t + 1],
                                scalar2=None,
                                op0=ADD,
                            )

                    return f

                def v_piece(i0):  # s-tiles i0, i0+1
                    def f():
                        pool = ytpp if (c == 0 and i0 == 2) else qkvp
                        tag = "ytps" if (c == 0 and i0 == 2) else "qv"
                        ps = pool.tile([P, CH], f32, tag=tag, name="vps")
                        psv = ps[:].rearrange("p (i f) -> p i f", f=P)
                        for i in (i0, i0 + 1):
                            for d in range(ND):
                                _lab("mm", f"v.c{c}.i{i}.d{d}")
                                nc.tensor.matmul(
                                    psv[:, i, :],
                                    xt_t(d)[:, i * P : (i + 1) * P],
                                    wv_sb[:, d, :],
                                    start=(i == i0 and d == 0),
                                    stop=(i == i0 + 1 and d == ND - 1),
                                )
                        for i in (i0, i0 + 1):
                            nc.vector.tensor_copy(
                                v_sbs[c][:, i, :, 0:64],
                                psv[:, i, :].rearrange("p (h f) -> p h f", f=HD),
                            )

                    return f

                return [qk_piece(0), qk_piece(1), v_piece(0), v_piece(2)]

            def emit_scores(j, tt):
                """QK^T for t-tile tt against s-chunk j, with causal mask."""
                k = tt - 4 * j
                o = P * k if k >= 0 else 0
                sc = scp.tile([P, HPC, CH], f32, tag="sc", name="sc")
                for h in range(HPC):
                    if USE_FP8 and j <= 1:
                        # early chunks read the natural-order bf16 Q/K tiles
                        def mm(a, b, start, stop):
                            _lab("mm", f"sc.j{j}.t{tt}.h{h}")
                            nc.tensor.matmul(
                                sc[:, h, a:b],
                                kt_sbs[tt // 4][
                                    64 * h : 64 * h + 64,
                                    (tt % 4) * P : (tt % 4 + 1) * P,
                                ],
                                qt_sbs[j][64 * h : 64 * h + 64, a:b],
                                start=start,
                                stop=stop,
                            )

                    elif USE_FP8:
                        lhsT = kt_drs[tt // 4][
                            32 * h : 32 * h + 32, :, (tt % 4) * P : (tt % 4 + 1) * P
                        ]

                        def mm(a, b, start, stop):
                            _lab("mm", f"sc.j{j}.t{tt}.h{h}")
                            nc.tensor.matmul(
                                sc[:, h, a:b],
                                lhsT,
                                qt_drs[j][32 * h : 32 * h + 32, :, a:b],
                                start=start,
                                stop=stop,
                                perf_mode=DR,
                            )
                    else:
                        lhsT = kt_sbs[tt // 4][
                            64 * h : 64 * h + 64, (tt % 4) * P : (tt % 4 + 1) * P
                        ]

                        def mm(a, b, start, stop):
                            _lab("mm", f"sc.j{j}.t{tt}.h{h}")
                            nc.tensor.matmul(
                                sc[:, h, a:b],
                                lhsT,
                                qt_sbs[j][64 * h : 64 * h + 64, a:b],
                                start=start,
                                stop=stop,
                            )

                    if k < 0:
                        mm(0, CH, True, True)
                    else:
                        # one group per bank: the first pass zeroes the whole
                        # 2KB zero-region, later passes accumulate
                        if o + P < CH:
                            mm(o + P, CH, True, False)
                            mm(o, o + P, False, False)
                        else:
                            mm(o, o + P, True, False)
                        # diagonal 128-block [o:o+P] gets the ramp mask added
                        _lab("mm", f"mask.j{j}.t{tt}.h{h}")
                        nc.tensor.matmul(
                            sc[:, h, o : o + P],
                            maskA_sb[:],
                            maskB_sb[:],
                            start=False,
                            stop=True,
                        )
                return sc, o

            def emit_exp(j, tt, sc, o):
                pt = ptp.tile([P, HPC, CH], bf16, tag="pt", name="pt")
                nc.scalar.activation(pt[:, :, o:], sc[:, :, o:], EXP, scale=0.125)
                return pt

            def emit_av(j, tt, pt):
                k = tt - 4 * j
                vt = v_sbs[tt // 4]
                for i in range(max(k, 0), 4):
                    for h in range(HPC):
                        _lab("mm", f"av.j{j}.t{tt}.i{i}.h{h}")
                        nc.tensor.matmul(
                            avs[i // 2][:, h, i % 2, 0:65],
                            pt[:, h, i * P : (i + 1) * P],
                            vt[:, tt % 4, h, :],
                            # one accumulation group per bank (zero region):
                            # first pass zeroes it, last pass closes it
                            start=(tt == 0 and i % 2 == 0 and h == 0),
                            stop=(tt == 4 * j + i and i % 2 == 1 and h == 1),
                        )

            def emit_div(j, i, otT, tail=False):
                av = avs[i // 2]
                rc = rcp.tile([P, HPC, 1], f32, tag="rc", name="rc")
                nc.vector.reciprocal(rc[:], av[:, :, i % 2, 64:65])
                ot = otp.tile([P, P], f32, tag="ot", name="ot")
                for h in range(HPC):
                    if False:
                        nc.scalar.mul(
                            ot[:, HD * h : HD * (h + 1)],
                            av[:, h, i % 2, 0:64],
                            rc[:, h, :],
                        )
                    else:
                        nc.vector.tensor_scalar(
                            out=ot[:, HD * h : HD * (h + 1)],
                            in0=av[:, h, i % 2, 0:64],
                            scalar1=rc[:, h, :],
                            scalar2=None,
                            op0=MULT,
                        )
                # PE-transpose (f32) into av's dead region, then copy out
                # to SBUF as bf16; avoids the serialized HWDGE path
                tp = av[:, 0, i % 2, 0:P]
                _lab("mm", f"tp.j{j}.i{i}")
                nc.tensor.transpose(tp, ot[:], ident_sb[:])
                nc.vector.tensor_copy(otT[:, i, :], tp)

            def make_outproj_pieces(j, otT):
                yt_sb = ytp.tile([P, ND, CH], bf16, tag="yt", name="yt")
                rhs = otT[:].rearrange("p i f -> p (i f)")

                def piece(e):
                    def f():
                        ps = ytpp.tile([P, CH], f32, tag="ytps", name="ytps")
                        _lab("mm", f"op.j{j}.e{e}")
                        nc.tensor.matmul(
                            ps[:], wo_sb[:, e, :], rhs, start=True, stop=True
                        )
                        nc.vector.tensor_copy(yt_sb[:, e, :], ps[:])
                        if e == ND - 1:
                            nc.sync.dma_start(
                                yt_d[:, j * CH : (j + 1) * CH].rearrange(
                                    "(e p) s -> p e s", p=P
                                ),
                                yt_sb[:],
                            )

                    return f

                return [piece(e) for e in range(ND)]

            op_q = []
            otTs = {}

            def emit_div_pair(jj, pair, otT, tail=False):
                for i in (2 * pair, 2 * pair + 1):
                    emit_div(jj, i, otT, tail=tail)

            tail_state = {}

            def emit_tail_half(jj, half, otT):
                a, b = half * 256, half * 256 + 256
                if half == 0:
                    tail_state["yt"] = ytp.tile(
                        [P, ND, CH], bf16, tag="yt", name="yt"
                    )
                yt_sb = tail_state["yt"]
                rhs = otT[:, 2 * half : 2 * half + 2, :].rearrange(
                    "p i f -> p (i f)"
                )
                for e in range(ND):
                    if e % 2 == 0:
                        ps = ytpp.tile([P, CH], f32, tag="ytps", name="ytt")
                    else:
                        ps = qkvp.tile([P, CH], f32, tag="qv", name="ytt")
                    _lab("mm", f"op.j{jj}.e{e}.h{half}")
                    nc.tensor.matmul(
                        ps[:, 0:256], wo_sb[:, e, :], rhs, start=True, stop=True
                    )
                    nc.vector.tensor_copy(yt_sb[:, e, a:b], ps[:, 0:256])
                nc.sync.dma_start(
                    yt_d[:, jj * CH + a : jj * CH + b].rearrange(
                        "(e p) s -> p e s", p=P
                    ),
                    yt_sb[:, :, a:b],
                )

            def handle_pop(j):
                jj, t0, p0_ = fifo.pop(0)
                emit_av(jj, t0, p0_)
                k0 = t0 - 4 * jj
                tail = jj == NCHUNK - 1
                if k0 == 1:
                    otTs[jj] = otTp.tile([P, 4, P], bf16, tag="otT", name="otT")
                    emit_div_pair(jj, 0, otTs[jj], tail=tail)
                    if tail:
                        emit_tail_half(jj, 0, otTs[jj])
                elif k0 == 3:
                    emit_div_pair(jj, 1, otTs[jj], tail=tail)
                    if tail:
                        emit_tail_half(jj, 1, otTs.pop(jj))
                    else:
                        op_q.extend(make_outproj_pieces(jj, otTs.pop(jj)))

            # ---- merged software-pipelined stream ----
            p0 = make_qkv_pieces(0)
            nc.sync.dma_start(
                w_first_sb[:, :, P : 2 * P],
                w_first_d[:, P : 2 * P].rearrange("(dt p) f -> p dt f", p=P),
            )
            nc.sync.dma_start(
                b_first_sb[:], b_first_d[:].rearrange("(c p) -> p c", p=P)
            )
            nc.sync.dma_start(
                wv_sb[:], wv_d[:].rearrange("(dt p) f -> p dt f", p=P)
            )
            p0[0]()
            p0[1]()
            # masks needed by the first diagonal exp (~10us in)
            nc.sync.dma_start(maskA_sb[:], maskA_d[:])
            nc.sync.dma_start(maskB_sb[:], maskB_d[:])
            nc.sync.dma_start(ident_sb[:], ident_d[:])
            # xt(1) prefetch must beat the cold-path weight loads below
            pieces = {1: p0[2:] + make_qkv_pieces(1)}
            if USE_FP8:
                nc.sync.dma_start(
                    wqk_sb[:], wqk_d[:].rearrange("(dt p) f -> p dt f", p=P)
                )
                nc.sync.dma_start(
                    bqk_sb[:], bqk_d[:].rearrange("(c p) -> p c", p=P)
                )
            nc.sync.dma_start(
                wo_sb[:], wo_d[:].rearrange("p (e f) -> p e f", f=P)
            )
            fifo = []
            for j in range(NCHUNK):
                ntt = 4 * j + 4
                cur = pieces.pop(j + 1, [])
                for tt in range(ntt):
                    sc, o = emit_scores(j, tt)
                    pt = emit_exp(j, tt, sc, o)
                    fifo.append((j, tt, pt))
                    if tt % 2 == 1 and cur:
                        cur.pop(0)()
                    if len(fifo) > (4 if j == 0 else 2):
                        handle_pop(j)
                    if tt == ntt // 2 and j + 2 < NCHUNK:
                        pieces[j + 2] = make_qkv_pieces(j + 2)
                    if tt >= 4 and op_q:
                        op_q.pop(0)()
                while cur:
                    cur.pop(0)()
            while fifo:
                handle_pop(NCHUNK - 1)
            while op_q:
                op_q.pop(0)()

    return nc


@functools.lru_cache(maxsize=1)
def _get_nc(S_arg=S):
    nc = build_nc()
    nc.compile()
    return nc


def make_in_maps(input, Wqkv, bqkv, Wo):
    x = np.asarray(input, dtype=np.float32).reshape(S, D)
    xt = np.ascontiguousarray(x.T).astype(BF16)
    Wqkv = np.asarray(Wqkv, dtype=np.float32)
    bqkv = np.asarray(bqkv, dtype=np.float32)
    Wo = np.asarray(Wo, dtype=np.float32)
    Wq, Wk, Wv = Wqkv[:, 0:D], Wqkv[:, D : 2 * D], Wqkv[:, 2 * D : 3 * D]
    bq, bk = bqkv[0:D], bqkv[D : 2 * D]

    r = np.arange(P)
    maskA = np.where(r[:, None] < r[None, :], np.float32(-1e30), np.float32(0))
    maskB = (r[:, None] >= r[None, :]).astype(np.float32)
    maskA = np.ascontiguousarray(maskA.astype(BF16))
    maskB = np.ascontiguousarray(maskB.astype(BF16))
    ident = np.ascontiguousarray(np.eye(P, dtype=np.float32))

    # fp8 DoubleRow wants [h0 d0-31 | h1 d0-31 | h0 d32-63 | h1 d32-63]
    perm = np.r_[0:32, 64:96, 32:64, 96:128] if USE_FP8 else np.arange(128)

    in_maps = []
    for c in range(NCORES):
        hs = [HPC * c + i for i in range(HPC)]

        def headcols(W):
            return np.concatenate([W[:, h * HD : (h + 1) * HD] for h in hs], 1)

        def headvec(b):
            return np.concatenate([b[h * HD : (h + 1) * HD] for h in hs], 0)

        wq0, wk0 = headcols(Wq), headcols(Wk)
        bq0, bk0 = headvec(bq), headvec(bk)
        wq, wk = wq0[:, perm], wk0[:, perm]
        bq_l, bk_l = bq0[perm], bk0[perm]
        in_maps.append(
            {
                "xt": xt,
                "wqk": np.ascontiguousarray(
                    np.concatenate([wq, wk], axis=1).astype(BF16)
                ),
                "wqk0": np.ascontiguousarray(
                    np.concatenate([wq0, wk0], axis=1).astype(BF16)
                ),
                "bqk0": np.ascontiguousarray(
                    np.concatenate([bq0, bk0], axis=0).astype(np.float32)
                ),
                "wv": np.ascontiguousarray(headcols(Wv).astype(BF16)),
                "wo": np.ascontiguousarray(
                    Wo[hs[0] * HD : hs[0] * HD + HPC * HD, :].astype(BF16)
                ),
                "bqk": np.ascontiguousarray(
                    np.concatenate([bq_l, bk_l], axis=0).astype(np.float32)
                ),
                "maskA": maskA,
                "maskB": maskB,
                "ident": ident,
            }
        )
    return in_maps


def kernel(input, Wqkv, bqkv, Wo, bo):
    from concourse.bass_utils import run_bass_kernel_spmd

    nc = _get_nc()
    in_maps = make_in_maps(input, Wqkv, bqkv, Wo)
    res = None
    last_exc = None
    for _attempt in range(3):  # transient NRT/device errors: retry
        try:
            res = run_bass_kernel_spmd(nc, in_maps, core_ids=list(range(NCORES)))
            break
        except Exception as e:  # noqa: BLE001
            last_exc = e
    if res is None:
        raise last_exc
    acc = np.zeros((D, S), np.float32)
    for r in res.results:
        acc += np.asarray(r["yt"], dtype=np.float32)
    y = np.ascontiguousarray(acc.T)
    bv = np.asarray(bqkv, np.float32)[2 * D : 3 * D]
    y += (bv @ np.asarray(Wo, np.float32) + np.asarray(bo, np.float32))[None, :]
    return y.reshape(1, S, D)


# revision 35
# speedup vs baseline: 1.0088x; 1.0088x over previous
"""Multi-head causal self-attention (B=1, S=4096, D=1024, H=16) on 8 TRN2
NeuronCores, tensor-parallel over heads (2 heads per core).

Engine plan (per core, modeled):
  - PE: QKV projection bf16 (Q^T/K^T in weight-stationary form, V in
    X-stationary form so V lands as [s, f] with no transposes), scores via
    fp8e4 DoubleRow (0.5 cyc/row), causal mask added in-PSUM by a ramp
    matmul (-1e30*max(t-s,0) = A^T B), AV with P^T stationary and [V|1]
    moving (N=65 per pass, denominator rides along as column 64), output
    projection bf16.
  - ACT: only the softmax exp (scale=1/8 folded in), one inst per t-tile.
  - DVE: psum->sbuf copies for Q/K (bias add + fp8 cast), V copies,
    reciprocal + per-partition divide (the [s, f] layout makes the
    denominator a per-partition scalar).
  - Pool: output-projection psum->sbuf copies.
  - SP: all DMAs, including the DMA-XBAR transpose that turns ot [s,f]
    into ot^T [f,s] for the output projection.

Dependency hygiene: the tile framework tracks deps at tile granularity, so
qt/kt/v live in per-chunk tiles, scores in per-t-tile pool tiles, and the
QKV work for chunk j+2 is emitted in pieces interleaved between the t-tiles
of chunk j so the PE always has independent work while ACT runs exp.

Host folds bv through Wo into the final bias (exact), sums the 8 bf16
partials in fp32, and adds bo.
"""

import sys

sys.path.insert(0, "/opt/trn_rl_repo")

import functools
import numpy as np
import ml_dtypes

D = 1024
H = 16
HD = 64
NCORES = 8
HPC = H // NCORES  # 2 heads per core
P = 128
CH = 512
S = 4096
NCHUNK = S // CH  # 8
ND = D // P  # 8
NT = S // P  # 32

USE_FP8 = True  # fp8e4 DoubleRow for the QK^T matmul

BF16 = ml_dtypes.bfloat16

# emission-order labels per (engine, opcode) for profiling
LABELS = {}


def _lab(key, s):
    LABELS.setdefault(key, []).append(s)


def build_nc():
    import concourse.bacc as bacc
    import concourse.mybir as mybir
    from concourse import tile

    f32 = mybir.dt.float32
    bf16 = mybir.dt.bfloat16
    f8 = mybir.dt.float8e4
    ADD = mybir.AluOpType.add
    MULT = mybir.AluOpType.mult
    EXP = mybir.ActivationFunctionType.Exp
    DR = mybir.MatmulPerfMode.DoubleRow

    nc = bacc.Bacc("TRN2", target_bir_lowering=False, debug=False)

    xt_d = nc.dram_tensor("xt", [D, S], bf16, kind="ExternalInput")
    wqk_d = nc.dram_tensor("wqk", [D, 2 * P], bf16, kind="ExternalInput")
    wqk0_d = nc.dram_tensor("wqk0", [D, 2 * P], bf16, kind="ExternalInput")
    bqk0_d = nc.dram_tensor("bqk0", [2 * P], f32, kind="ExternalInput")
    wv_d = nc.dram_tensor("wv", [D, P], bf16, kind="ExternalInput")
    wo_d = nc.dram_tensor("wo", [P, D], bf16, kind="ExternalInput")
    bqk_d = nc.dram_tensor("bqk", [2 * P], f32, kind="ExternalInput")
    maskA_d = nc.dram_tensor("maskA", [P, P], bf16, kind="ExternalInput")
    maskB_d = nc.dram_tensor("maskB", [P, P], bf16, kind="ExternalInput")
    ident_d = nc.dram_tensor("ident", [P, P], f32, kind="ExternalInput")
    yt_d = nc.dram_tensor("yt", [D, S], bf16, kind="ExternalOutput")

    with tile.TileContext(nc) as tc:
        with (
            tc.tile_pool(name="consts", bufs=1) as consts,
            tc.tile_pool(name="xtp", bufs=6) as xtp,
            tc.tile_pool(name="f8p", bufs=4) as f8p,
            tc.tile_pool(name="ptp", bufs=36) as ptp,
            tc.tile_pool(name="otp", bufs=6) as otp,
            tc.tile_pool(name="otTp", bufs=2) as otTp,
            tc.tile_pool(name="rcp", bufs=4) as rcp,
            tc.tile_pool(name="ytp", bufs=2) as ytp,
            tc.tile_pool(name="scp", bufs=2, space="PSUM") as scp,
            tc.tile_pool(name="avp", bufs=1, space="PSUM") as avp,
            tc.tile_pool(name="qkvp", bufs=1, space="PSUM") as qkvp,
            tc.tile_pool(name="ytpp", bufs=1, space="PSUM") as ytpp,
        ):
            # ---- persistent SBUF ----
            wqk_sb = consts.tile([P, ND, 2 * P], bf16)
            wv_sb = consts.tile([P, ND, P], bf16)
            wo_sb = consts.tile([P, ND, P], bf16)
            bqk_sb = consts.tile([P, 2], f32)
            wqk0_sb = consts.tile([P, ND, 2 * P], bf16)
            bqk0_sb = consts.tile([P, 2], f32)
            maskA_sb = consts.tile([P, P], bf16)
            maskB_sb = consts.tile([P, P], bf16)
            ident_sb = consts.tile([P, P], f32)
            # per-chunk tensors (separate tiles -> precise deps)
            v_sbs = [
                consts.tile([P, 4, HPC, 65], bf16, name=f"v{c}")
                for c in range(NCHUNK)
            ]
            if USE_FP8:
                qt_drs = [
                    consts.tile([64, 2, CH], f8, name=f"qdr{c}")
                    for c in range(NCHUNK)
                ]
                kt_drs = [
                    consts.tile([64, 2, CH], f8, name=f"kdr{c}")
                    for c in range(NCHUNK)
                ]
                # chunks 0-1 also get bf16 Q/K so the early scores skip
                # the fp8 rearrange round-trip (startup critical path)
                qt_sbs = [
                    consts.tile([P, CH], bf16, name=f"q{c}") for c in range(3)
                ]
                kt_sbs = [
                    consts.tile([P, CH], bf16, name=f"k{c}") for c in range(3)
                ]
            else:
                qt_sbs = [
                    consts.tile([P, CH], bf16, name=f"q{c}")
                    for c in range(NCHUNK)
                ]
                kt_sbs = [
                    consts.tile([P, CH], bf16, name=f"k{c}")
                    for c in range(NCHUNK)
                ]

            w_first_sb = wqk0_sb if USE_FP8 else wqk_sb
            w_first_d = wqk0_d if USE_FP8 else wqk_d
            b_first_sb = bqk0_sb if USE_FP8 else bqk_sb
            b_first_d = bqk0_d if USE_FP8 else bqk_d
            nc.sync.dma_start(
                w_first_sb[:, :, 0:P],
                w_first_d[:, 0:P].rearrange("(dt p) f -> p dt f", p=P),
            )
            for c in range(NCHUNK):
                nc.gpsimd.memset(v_sbs[c][:, :, :, 64:65], 1.0)

            # AV accumulators: [s, head, stile pair, 0:65 used]
            # two 1-bank tiles so early stile divs don't serialize later AVs
            avs = [
                avp.tile([P, HPC, 2, P], f32, name=f"av{b}") for b in range(2)
            ]

            def make_qkv_pieces(c):
                """QKV for chunk c as a list of emit-closures.

                X^T arrives in two half-tiles so the first Q matmuls can
                start as soon as the first half lands.
                """
                xt_h = [
                    xtp.tile([P, ND // 2, CH], bf16, tag="xt", name="xt")
                    for _ in range(2)
                ]
                for g in range(2):
                    nc.sync.dma_start(
                        xt_h[g][:],
                        xt_d[
                            g * (D // 2) : (g + 1) * (D // 2),
                            c * CH : (c + 1) * CH,
                        ].rearrange("(dt p) s -> p dt s", p=P),
                    )

                def xt_t(d):
                    return xt_h[d // 4][:, d % 4]

                def qk_piece(t):  # 0 = Q, 1 = K
                    def f():
                        natural = USE_FP8 and c <= 1
                        w_sb = wqk0_sb if natural else wqk_sb
                        b_sb = bqk0_sb if natural else bqk_sb
                        pool = ytpp if (c == 0 and t == 1) else qkvp
                        tag = "ytps" if (c == 0 and t == 1) else "qv"
                        ps = pool.tile([P, CH], f32, tag=tag, name=f"qk{t}")
                        for d in range(ND):
                            _lab("mm", f"qk{t}.c{c}.d{d}")
                            nc.tensor.matmul(
                                ps[:],
                                w_sb[:, d, t * P : (t + 1) * P],
                                xt_t(d)[:, :],
                                start=(d == 0),
                                stop=(d == ND - 1),
                            )
                        if USE_FP8:
                            if natural:
                                nc.vector.tensor_scalar(
                                    out=[qt_sbs, kt_sbs][t][c][:],
                                    in0=ps[:],
                                    scalar1=b_sb[:, t : t + 1],
                                    scalar2=None,
                                    op0=ADD,
                                )
                            st = f8p.tile([P, CH], f8, tag="f8", name="f8st")
                            nc.vector.tensor_scalar(
                                out=st[:],
                                in0=ps[:],
                                scalar1=b_sb[:, t : t + 1],
                                scalar2=None,
                                op0=ADD,
                            )
                            dr_t = [qt_drs, kt_drs][t][c]
                            if natural:
                                # natural [h0|h1] order -> dr layout via 4
                                # transfers (off the startup critical path)
                                for h in range(HPC):
                                    for i in range(2):
                                        nc.sync.dma_start(
                                            dr_t[32 * h : 32 * h + 32, i, :],
                                            st[
                                                64 * h + 32 * i : 64 * h
                                                + 32 * i
                                                + 32,
                                                :,
                                            ],
                                        )
                            else:
                                nc.sync.dma_start(
                                    dr_t[0:64, 0, :], st[0:64, :]
                                )
                                nc.sync.dma_start(
                                    dr_t[0:64, 1, :], st[64:128, :]
                                )
                        else:
                            dest = [qt_sbs, kt_sbs][t][c]
                            nc.vector.tensor_scalar(
                                out=dest[:],
                                in0=ps[:],
                                scalar1=bqk_sb[:, t : t + 1],
                                scalar2=None,
                                op0=ADD,
                            )

                    return f

                def v_piece(i0):  # s-tiles i0, i0+1
                    def f():
                        pool = ytpp if (c == 0 and i0 == 2) else qkvp
                        tag = "ytps" if (c == 0 and i0 == 2) else "qv"
                        ps = pool.tile([P, CH], f32, tag=tag, name="vps")
                        psv = ps[:].rearrange("p (i f) -> p i f", f=P)
                        for i in (i0, i0 + 1):
                            for d in range(ND):
                                _lab("mm", f"v.c{c}.i{i}.d{d}")
                                nc.tensor.matmul(
                                    psv[:, i, :],
                                    xt_t(d)[:, i * P : (i + 1) * P],
                                    wv_sb[:, d, :],
                                    start=(i == i0 and d == 0),
                                    stop=(i == i0 + 1 and d == ND - 1),
                                )
                        for i in (i0, i0 + 1):
                            nc.vector.tensor_copy(
                                v_sbs[c][:, i, :, 0:64],
                                psv[:, i, :].rearrange("p (h f) -> p h f", f=HD),
                            )

                    return f

                return [qk_piece(0), qk_piece(1), v_piece(0), v_piece(2)]

            def emit_scores(j, tt):
                """QK^T for t-tile tt against s-chunk j, with causal mask."""
                k = tt - 4 * j
                o = P * k if k >= 0 else 0
                sc = scp.tile([P, HPC, CH], f32, tag="sc", name="sc")
                for h in range(HPC):
                    if USE_FP8 and j <= 1:
                        # early chunks read the natural-order bf16 Q/K tiles
                        def mm(a, b, start, stop):
                            _lab("mm", f"sc.j{j}.t{tt}.h{h}")
                            nc.tensor.matmul(
                                sc[:, h, a:b],
                                kt_sbs[tt // 4][
                                    64 * h : 64 * h + 64,
                                    (tt % 4) * P : (tt % 4 + 1) * P,
                                ],
                                qt_sbs[j][64 * h : 64 * h + 64, a:b],
                                start=start,
                                stop=stop,
                            )

                    elif USE_FP8:
                        lhsT = kt_drs[tt // 4][
                            32 * h : 32 * h + 32, :, (tt % 4) * P : (tt % 4 + 1) * P
                        ]

                        def mm(a, b, start, stop):
                            _lab("mm", f"sc.j{j}.t{tt}.h{h}")
                            nc.tensor.matmul(
                                sc[:, h, a:b],
                                lhsT,
                                qt_drs[j][32 * h : 32 * h + 32, :, a:b],
                                start=start,
                                stop=stop,
                                perf_mode=DR,
                            )
                    else:
                        lhsT = kt_sbs[tt // 4][
                            64 * h : 64 * h + 64, (tt % 4) * P : (tt % 4 + 1) * P
                        ]

                        def mm(a, b, start, stop):
                            _lab("mm", f"sc.j{j}.t{tt}.h{h}")
                            nc.tensor.matmul(
                                sc[:, h, a:b],
                                lhsT,
                                qt_sbs[j][64 * h : 64 * h + 64, a:b],
                                start=start,
                                stop=stop,
                            )

                    if k < 0:
                        mm(0, CH, True, True)
                    else:
                        # one group per bank: the first pass zeroes the whole
                        # 2KB zero-region, later passes accumulate
                        if o + P < CH:
                            mm(o + P, CH, True, False)
                            mm(o, o + P, False, False)
                        else:
                            mm(o, o + P, True, False)
                        # diagonal 128-block [o:o+P] gets the ramp mask added
                        _lab("mm", f"mask.j{j}.t{tt}.h{h}")
                        nc.tensor.matmul(
                            sc[:, h, o : o + P],
                            maskA_sb[:],
                            maskB_sb[:],
                            start=False,
                            stop=True,
                        )
                return sc, o

            def emit_exp(j, tt, sc, o):
                pt = ptp.tile([P, HPC, CH], bf16, tag="pt", name="pt")
                nc.scalar.activation(pt[:, :, o:], sc[:, :, o:], EXP, scale=0.125)
                return pt

            def emit_av(j, tt, pt):
                k = tt - 4 * j
                vt = v_sbs[tt // 4]
                for i in range(max(k, 0), 4):
                    for h in range(HPC):
                        _lab("mm", f"av.j{j}.t{tt}.i{i}.h{h}")
                        nc.tensor.matmul(
                            avs[i // 2][:, h, i % 2, 0:65],
                            pt[:, h, i * P : (i + 1) * P],
                            vt[:, tt % 4, h, :],
                            # one accumulation group per bank (zero region):
                            # first pass zeroes it, last pass closes it
                            start=(tt == 0 and i % 2 == 0 and h == 0),
                            stop=(tt == 4 * j + i and i % 2 == 1 and h == 1),
                        )

            def emit_div(j, i, otT, tail=False):
                av = avs[i // 2]
                rc = rcp.tile([P, HPC, 1], f32, tag="rc", name="rc")
                nc.vector.reciprocal(rc[:], av[:, :, i % 2, 64:65])
                ot = otp.tile([P, P], f32, tag="ot", name="ot")
                for h in range(HPC):
                    if False:
                        nc.scalar.mul(
                            ot[:, HD * h : HD * (h + 1)],
                            av[:, h, i % 2, 0:64],
                            rc[:, h, :],
                        )
                    else:
                        nc.vector.tensor_scalar(
                            out=ot[:, HD * h : HD * (h + 1)],
                            in0=av[:, h, i % 2, 0:64],
                            scalar1=rc[:, h, :],
                            scalar2=None,
                            op0=MULT,
                        )
                # PE-transpose (f32) into av's dead region, then copy out
                # to SBUF as bf16; avoids the serialized HWDGE path
                tp = av[:, 0, i % 2, 0:P]
                _lab("mm", f"tp.j{j}.i{i}")
                nc.tensor.transpose(tp, ot[:], ident_sb[:])
                nc.vector.tensor_copy(otT[:, i, :], tp)

            def make_outproj_pieces(j, otT):
                yt_sb = ytp.tile([P, ND, CH], bf16, tag="yt", name="yt")
                rhs = otT[:].rearrange("p i f -> p (i f)")

                def piece(e):
                    def f():
                        ps = ytpp.tile([P, CH], f32, tag="ytps", name="ytps")
                        _lab("mm", f"op.j{j}.e{e}")
                        nc.tensor.matmul(
                            ps[:], wo_sb[:, e, :], rhs, start=True, stop=True
                        )
                        nc.vector.tensor_copy(yt_sb[:, e, :], ps[:])
                        if e == ND - 1:
                            nc.sync.dma_start(
                                yt_d[:, j * CH : (j + 1) * CH].rearrange(
                                    "(e p) s -> p e s", p=P
                                ),
                                yt_sb[:],
                            )

                    return f

                return [piece(e) for e in range(ND)]

            op_q = []
            otTs = {}

            def emit_div_pair(jj, pair, otT, tail=False):
                for i in (2 * pair, 2 * pair + 1):
                    emit_div(jj, i, otT, tail=tail)

            tail_state = {}

            def emit_tail_half(jj, half, otT):
                a, b = half * 256, half * 256 + 256
                if half == 0:
                    tail_state["yt"] = ytp.tile(
                        [P, ND, CH], bf16, tag="yt", name="yt"
                    )
                yt_sb = tail_state["yt"]
                rhs = otT[:, 2 * half : 2 * half + 2, :].rearrange(
                    "p i f -> p (i f)"
                )
                for e in range(ND):
                    if e % 2 == 0:
                        ps = ytpp.tile([P, CH], f32, tag="ytps", name="ytt")
                    else:
                        ps = qkvp.tile([P, CH], f32, tag="qv", name="ytt")
                    _lab("mm", f"op.j{jj}.e{e}.h{half}")
                    nc.tensor.matmul(
                        ps[:, 0:256], wo_sb[:, e, :], rhs, start=True, stop=True
                    )
                    if e % 2 == 0:
                        nc.scalar.copy(yt_sb[:, e, a:b], ps[:, 0:256])
                    else:
                        nc.vector.tensor_copy(yt_sb[:, e, a:b], ps[:, 0:256])
                nc.sync.dma_start(
                    yt_d[:, jj * CH + a : jj * CH + b].rearrange(
                        "(e p) s -> p e s", p=P
                    ),
                    yt_sb[:, :, a:b],
                )

            def handle_pop(j):
                jj, t0, p0_ = fifo.pop(0)
                emit_av(jj, t0, p0_)
                k0 = t0 - 4 * jj
                tail = jj == NCHUNK - 1
                if k0 == 1:
                    otTs[jj] = otTp.tile([P, 4, P], bf16, tag="otT", name="otT")
                    emit_div_pair(jj, 0, otTs[jj], tail=tail)
                    if tail:
                        emit_tail_half(jj, 0, otTs[jj])
                elif k0 == 3:
                    emit_div_pair(jj, 1, otTs[jj], tail=tail)
                    if tail:
                        emit_tail_half(jj, 1, otTs.pop(jj))
                    else:
                        op_q.extend(make_outproj_pieces(jj, otTs.pop(jj)))

            # ---- merged software-pipelined stream ----
            p0 = make_qkv_pieces(0)
            nc.sync.dma_start(
                w_first_sb[:, :, P : 2 * P],
                w_first_d[:, P : 2 * P].rearrange("(dt p) f -> p dt f", p=P),
            )
            nc.sync.dma_start(
                b_first_sb[:], b_first_d[:].rearrange("(c p) -> p c", p=P)
            )
            nc.sync.dma_start(
                wv_sb[:], wv_d[:].rearrange("(dt p) f -> p dt f", p=P)
            )
            p0[0]()
            p0[1]()
            # masks needed by the first diagonal exp (~10us in)
            nc.sync.dma_start(maskA_sb[:], maskA_d[:])
            nc.sync.dma_start(maskB_sb[:], maskB_d[:])
            nc.sync.dma_start(ident_sb[:], ident_d[:])
            # xt(1) prefetch must beat the cold-path weight loads below
            pieces = {1: p0[2:] + make_qkv_pieces(1)}
            if USE_FP8:
                nc.sync.dma_start(
                    wqk_sb[:], wqk_d[:].rearrange("(dt p) f -> p dt f", p=P)
                )
                nc.sync.dma_start(
                    bqk_sb[:], bqk_d[:].rearrange("(c p) -> p c", p=P)
                )
            nc.sync.dma_start(
                wo_sb[:], wo_d[:].rearrange("p (e f) -> p e f", f=P)
            )
            fifo = []
            for j in range(NCHUNK):
                ntt = 4 * j + 4
                cur = pieces.pop(j + 1, [])
                for tt in range(ntt):
                    sc, o = emit_scores(j, tt)
                    pt = emit_exp(j, tt, sc, o)
                    fifo.append((j, tt, pt))
                    if tt % 2 == 1 and cur:
                        cur.pop(0)()
                    if len(fifo) > (4 if j == 0 else 2):
                        handle_pop(j)
                    if tt == ntt // 2 and j + 2 < NCHUNK:
                        pieces[j + 2] = make_qkv_pieces(j + 2)
                    if tt >= 4 and op_q:
                        op_q.pop(0)()
                while cur:
                    cur.pop(0)()
            while fifo:
                handle_pop(NCHUNK - 1)
            while op_q:
                op_q.pop(0)()

    return nc


@functools.lru_cache(maxsize=1)
def _get_nc(S_arg=S):
    nc = build_nc()
    nc.compile()
    return nc


def make_in_maps(input, Wqkv, bqkv, Wo):
    x = np.asarray(input, dtype=np.float32).reshape(S, D)
    xt = np.ascontiguousarray(x.T).astype(BF16)
    Wqkv = np.asarray(Wqkv, dtype=np.float32)
    bqkv = np.asarray(bqkv, dtype=np.float32)
    Wo = np.asarray(Wo, dtype=np.float32)
    Wq, Wk, Wv = Wqkv[:, 0:D], Wqkv[:, D : 2 * D], Wqkv[:, 2 * D : 3 * D]
    bq, bk = bqkv[0:D], bqkv[D : 2 * D]

    r = np.arange(P)
    maskA = np.where(r[:, None] < r[None, :], np.float32(-1e30), np.float32(0))
    maskB = (r[:, None] >= r[None, :]).astype(np.float32)
    maskA = np.ascontiguousarray(maskA.astype(BF16))
    maskB = np.ascontiguousarray(maskB.astype(BF16))
    ident = np.ascontiguousarray(np.eye(P, dtype=np.float32))

    # fp8 DoubleRow wants [h0 d0-31 | h1 d0-31 | h0 d32-63 | h1 d32-63]
    perm = np.r_[0:32, 64:96, 32:64, 96:128] if USE_FP8 else np.arange(128)

    in_maps = []
    for c in range(NCORES):
        hs = [HPC * c + i for i in range(HPC)]

        def headcols(W):
            return np.concatenate([W[:, h * HD : (h + 1) * HD] for h in hs], 1)

        def headvec(b):
            return np.concatenate([b[h * HD : (h + 1) * HD] for h in hs], 0)

        wq0, wk0 = headcols(Wq), headcols(Wk)
        bq0, bk0 = headvec(bq), headvec(bk)
        wq, wk = wq0[:, perm], wk0[:, perm]
        bq_l, bk_l = bq0[perm], bk0[perm]
        in_maps.append(
            {
                "xt": xt,
                "wqk": np.ascontiguousarray(
                    np.concatenate([wq, wk], axis=1).astype(BF16)
                ),
                "wqk0": np.ascontiguousarray(
                    np.concatenate([wq0, wk0], axis=1).astype(BF16)
                ),
                "bqk0": np.ascontiguousarray(
                    np.concatenate([bq0, bk0], axis=0).astype(np.float32)
                ),
                "wv": np.ascontiguousarray(headcols(Wv).astype(BF16)),
                "wo": np.ascontiguousarray(
                    Wo[hs[0] * HD : hs[0] * HD + HPC * HD, :].astype(BF16)
                ),
                "bqk": np.ascontiguousarray(
                    np.concatenate([bq_l, bk_l], axis=0).astype(np.float32)
                ),
                "maskA": maskA,
                "maskB": maskB,
                "ident": ident,
            }
        )
    return in_maps


def kernel(input, Wqkv, bqkv, Wo, bo):
    from concourse.bass_utils import run_bass_kernel_spmd

    nc = _get_nc()
    in_maps = make_in_maps(input, Wqkv, bqkv, Wo)
    res = None
    last_exc = None
    for _attempt in range(3):  # transient NRT/device errors: retry
        try:
            res = run_bass_kernel_spmd(nc, in_maps, core_ids=list(range(NCORES)))
            break
        except Exception as e:  # noqa: BLE001
            last_exc = e
    if res is None:
        raise last_exc
    acc = np.zeros((D, S), np.float32)
    for r in res.results:
        acc += np.asarray(r["yt"], dtype=np.float32)
    y = np.ascontiguousarray(acc.T)
    bv = np.asarray(bqkv, np.float32)[2 * D : 3 * D]
    y += (bv @ np.asarray(Wo, np.float32) + np.asarray(bo, np.float32))[None, :]
    return y.reshape(1, S, D)


# revision 36
# speedup vs baseline: 1.0186x; 1.0097x over previous
"""Multi-head causal self-attention (B=1, S=4096, D=1024, H=16) on 8 TRN2
NeuronCores, tensor-parallel over heads (2 heads per core).

Engine plan (per core, modeled):
  - PE: QKV projection bf16 (Q^T/K^T in weight-stationary form, V in
    X-stationary form so V lands as [s, f] with no transposes), scores via
    fp8e4 DoubleRow (0.5 cyc/row), causal mask added in-PSUM by a ramp
    matmul (-1e30*max(t-s,0) = A^T B), AV with P^T stationary and [V|1]
    moving (N=65 per pass, denominator rides along as column 64), output
    projection bf16.
  - ACT: only the softmax exp (scale=1/8 folded in), one inst per t-tile.
  - DVE: psum->sbuf copies for Q/K (bias add + fp8 cast), V copies,
    reciprocal + per-partition divide (the [s, f] layout makes the
    denominator a per-partition scalar).
  - Pool: output-projection psum->sbuf copies.
  - SP: all DMAs, including the DMA-XBAR transpose that turns ot [s,f]
    into ot^T [f,s] for the output projection.

Dependency hygiene: the tile framework tracks deps at tile granularity, so
qt/kt/v live in per-chunk tiles, scores in per-t-tile pool tiles, and the
QKV work for chunk j+2 is emitted in pieces interleaved between the t-tiles
of chunk j so the PE always has independent work while ACT runs exp.

Host folds bv through Wo into the final bias (exact), sums the 8 bf16
partials in fp32, and adds bo.
"""

import sys

sys.path.insert(0, "/opt/trn_rl_repo")

import functools
import numpy as np
import ml_dtypes

D = 1024
H = 16
HD = 64
NCORES = 8
HPC = H // NCORES  # 2 heads per core
P = 128
CH = 512
S = 4096
NCHUNK = S // CH  # 8
ND = D // P  # 8
NT = S // P  # 32

USE_FP8 = True  # fp8e4 DoubleRow for the QK^T matmul

BF16 = ml_dtypes.bfloat16

# emission-order labels per (engine, opcode) for profiling
LABELS = {}


def _lab(key, s):
    LABELS.setdefault(key, []).append(s)


def build_nc():
    import concourse.bacc as bacc
    import concourse.mybir as mybir
    from concourse import tile

    f32 = mybir.dt.float32
    bf16 = mybir.dt.bfloat16
    f8 = mybir.dt.float8e4
    ADD = mybir.AluOpType.add
    MULT = mybir.AluOpType.mult
    EXP = mybir.ActivationFunctionType.Exp
    DR = mybir.MatmulPerfMode.DoubleRow

    nc = bacc.Bacc("TRN2", target_bir_lowering=False, debug=False)

    xt_d = nc.dram_tensor("xt", [D, S], bf16, kind="ExternalInput")
    wqk_d = nc.dram_tensor("wqk", [D, 2 * P], bf16, kind="ExternalInput")
    wqk0_d = nc.dram_tensor("wqk0", [D, 2 * P], bf16, kind="ExternalInput")
    bqk0_d = nc.dram_tensor("bqk0", [2 * P], f32, kind="ExternalInput")
    wv_d = nc.dram_tensor("wv", [D, P], bf16, kind="ExternalInput")
    wo_d = nc.dram_tensor("wo", [P, D], bf16, kind="ExternalInput")
    bqk_d = nc.dram_tensor("bqk", [2 * P], f32, kind="ExternalInput")
    maskA_d = nc.dram_tensor("maskA", [P, P], bf16, kind="ExternalInput")
    maskB_d = nc.dram_tensor("maskB", [P, P], bf16, kind="ExternalInput")
    ident_d = nc.dram_tensor("ident", [P, P], f32, kind="ExternalInput")
    yt_d = nc.dram_tensor("yt", [D, S], bf16, kind="ExternalOutput")

    with tile.TileContext(nc) as tc:
        with (
            tc.tile_pool(name="consts", bufs=1) as consts,
            tc.tile_pool(name="xtp", bufs=6) as xtp,
            tc.tile_pool(name="f8p", bufs=4) as f8p,
            tc.tile_pool(name="ptp", bufs=36) as ptp,
            tc.tile_pool(name="otp", bufs=6) as otp,
            tc.tile_pool(name="otTp", bufs=2) as otTp,
            tc.tile_pool(name="rcp", bufs=4) as rcp,
            tc.tile_pool(name="ytp", bufs=2) as ytp,
            tc.tile_pool(name="scp", bufs=2, space="PSUM") as scp,
            tc.tile_pool(name="avp", bufs=1, space="PSUM") as avp,
            tc.tile_pool(name="qkvp", bufs=1, space="PSUM") as qkvp,
            tc.tile_pool(name="ytpp", bufs=1, space="PSUM") as ytpp,
        ):
            # ---- persistent SBUF ----
            wqk_sb = consts.tile([P, ND, 2 * P], bf16)
            wv_sb = consts.tile([P, ND, P], bf16)
            wo_sb = consts.tile([P, ND, P], bf16)
            bqk_sb = consts.tile([P, 2], f32)
            wqk0_sb = consts.tile([P, ND, 2 * P], bf16)
            bqk0_sb = consts.tile([P, 2], f32)
            maskA_sb = consts.tile([P, P], bf16)
            maskB_sb = consts.tile([P, P], bf16)
            ident_sb = consts.tile([P, P], f32)
            # per-chunk tensors (separate tiles -> precise deps)
            v_sbs = [
                consts.tile([P, 4, HPC, 65], bf16, name=f"v{c}")
                for c in range(NCHUNK)
            ]
            if USE_FP8:
                qt_drs = [
                    consts.tile([64, 2, CH], f8, name=f"qdr{c}")
                    for c in range(NCHUNK)
                ]
                kt_drs = [
                    consts.tile([64, 2, CH], f8, name=f"kdr{c}")
                    for c in range(NCHUNK)
                ]
                # chunks 0-1 also get bf16 Q/K so the early scores skip
                # the fp8 rearrange round-trip (startup critical path)
                qt_sbs = [
                    consts.tile([P, CH], bf16, name=f"q{c}") for c in range(3)
                ]
                kt_sbs = [
                    consts.tile([P, CH], bf16, name=f"k{c}") for c in range(3)
                ]
            else:
                qt_sbs = [
                    consts.tile([P, CH], bf16, name=f"q{c}")
                    for c in range(NCHUNK)
                ]
                kt_sbs = [
                    consts.tile([P, CH], bf16, name=f"k{c}")
                    for c in range(NCHUNK)
                ]

            w_first_sb = wqk0_sb if USE_FP8 else wqk_sb
            w_first_d = wqk0_d if USE_FP8 else wqk_d
            b_first_sb = bqk0_sb if USE_FP8 else bqk_sb
            b_first_d = bqk0_d if USE_FP8 else bqk_d
            nc.sync.dma_start(
                w_first_sb[:, :, 0:P],
                w_first_d[:, 0:P].rearrange("(dt p) f -> p dt f", p=P),
            )
            for c in range(NCHUNK):
                nc.gpsimd.memset(v_sbs[c][:, :, :, 64:65], 1.0)

            # AV accumulators: [s, head, stile pair, 0:65 used]
            # two 1-bank tiles so early stile divs don't serialize later AVs
            avs = [
                avp.tile([P, HPC, 2, P], f32, name=f"av{b}") for b in range(2)
            ]

            def make_qkv_pieces(c):
                """QKV for chunk c as a list of emit-closures.

                X^T arrives in two half-tiles so the first Q matmuls can
                start as soon as the first half lands.
                """
                xt_h = [
                    xtp.tile([P, ND // 2, CH], bf16, tag="xt", name="xt")
                    for _ in range(2)
                ]
                for g in range(2):
                    nc.sync.dma_start(
                        xt_h[g][:],
                        xt_d[
                            g * (D // 2) : (g + 1) * (D // 2),
                            c * CH : (c + 1) * CH,
                        ].rearrange("(dt p) s -> p dt s", p=P),
                    )

                def xt_t(d):
                    return xt_h[d // 4][:, d % 4]

                def qk_piece(t):  # 0 = Q, 1 = K
                    def f():
                        natural = USE_FP8 and c <= 1
                        w_sb = wqk0_sb if natural else wqk_sb
                        b_sb = bqk0_sb if natural else bqk_sb
                        pool = ytpp if (c == 0 and t == 1) else qkvp
                        tag = "ytps" if (c == 0 and t == 1) else "qv"
                        ps = pool.tile([P, CH], f32, tag=tag, name=f"qk{t}")
                        for d in range(ND):
                            _lab("mm", f"qk{t}.c{c}.d{d}")
                            nc.tensor.matmul(
                                ps[:],
                                w_sb[:, d, t * P : (t + 1) * P],
                                xt_t(d)[:, :],
                                start=(d == 0),
                                stop=(d == ND - 1),
                            )
                        if USE_FP8:
                            if natural:
                                nc.vector.tensor_scalar(
                                    out=[qt_sbs, kt_sbs][t][c][:],
                                    in0=ps[:],
                                    scalar1=b_sb[:, t : t + 1],
                                    scalar2=None,
                                    op0=ADD,
                                )
                            st = f8p.tile([P, CH], f8, tag="f8", name="f8st")
                            nc.vector.tensor_scalar(
                                out=st[:],
                                in0=ps[:],
                                scalar1=b_sb[:, t : t + 1],
                                scalar2=None,
                                op0=ADD,
                            )
                            dr_t = [qt_drs, kt_drs][t][c]
                            if natural:
                                # natural [h0|h1] order -> dr layout via 4
                                # transfers (off the startup critical path)
                                for h in range(HPC):
                                    for i in range(2):
                                        nc.sync.dma_start(
                                            dr_t[32 * h : 32 * h + 32, i, :],
                                            st[
                                                64 * h + 32 * i : 64 * h
                                                + 32 * i
                                                + 32,
                                                :,
                                            ],
                                        )
                            else:
                                nc.sync.dma_start(
                                    dr_t[0:64, 0, :], st[0:64, :]
                                )
                                nc.sync.dma_start(
                                    dr_t[0:64, 1, :], st[64:128, :]
                                )
                        else:
                            dest = [qt_sbs, kt_sbs][t][c]
                            nc.vector.tensor_scalar(
                                out=dest[:],
                                in0=ps[:],
                                scalar1=bqk_sb[:, t : t + 1],
                                scalar2=None,
                                op0=ADD,
                            )

                    return f

                def v_piece(i0):  # s-tiles i0, i0+1
                    def f():
                        pool = ytpp if (c == 0 and i0 == 2) else qkvp
                        tag = "ytps" if (c == 0 and i0 == 2) else "qv"
                        ps = pool.tile([P, CH], f32, tag=tag, name="vps")
                        psv = ps[:].rearrange("p (i f) -> p i f", f=P)
                        for i in (i0, i0 + 1):
                            for d in range(ND):
                                _lab("mm", f"v.c{c}.i{i}.d{d}")
                                nc.tensor.matmul(
                                    psv[:, i, :],
                                    xt_t(d)[:, i * P : (i + 1) * P],
                                    wv_sb[:, d, :],
                                    start=(i == i0 and d == 0),
                                    stop=(i == i0 + 1 and d == ND - 1),
                                )
                        for i in (i0, i0 + 1):
                            nc.vector.tensor_copy(
                                v_sbs[c][:, i, :, 0:64],
                                psv[:, i, :].rearrange("p (h f) -> p h f", f=HD),
                            )

                    return f

                return [qk_piece(0), qk_piece(1), v_piece(0), v_piece(2)]

            def emit_scores(j, tt):
                """QK^T for t-tile tt against s-chunk j, with causal mask."""
                k = tt - 4 * j
                o = P * k if k >= 0 else 0
                sc = scp.tile([P, HPC, CH], f32, tag="sc", name="sc")
                for h in range(HPC):
                    if USE_FP8 and j <= 1:
                        # early chunks read the natural-order bf16 Q/K tiles
                        def mm(a, b, start, stop):
                            _lab("mm", f"sc.j{j}.t{tt}.h{h}")
                            nc.tensor.matmul(
                                sc[:, h, a:b],
                                kt_sbs[tt // 4][
                                    64 * h : 64 * h + 64,
                                    (tt % 4) * P : (tt % 4 + 1) * P,
                                ],
                                qt_sbs[j][64 * h : 64 * h + 64, a:b],
                                start=start,
                                stop=stop,
                            )

                    elif USE_FP8:
                        lhsT = kt_drs[tt // 4][
                            32 * h : 32 * h + 32, :, (tt % 4) * P : (tt % 4 + 1) * P
                        ]

                        def mm(a, b, start, stop):
                            _lab("mm", f"sc.j{j}.t{tt}.h{h}")
                            nc.tensor.matmul(
                                sc[:, h, a:b],
                                lhsT,
                                qt_drs[j][32 * h : 32 * h + 32, :, a:b],
                                start=start,
                                stop=stop,
                                perf_mode=DR,
                            )
                    else:
                        lhsT = kt_sbs[tt // 4][
                            64 * h : 64 * h + 64, (tt % 4) * P : (tt % 4 + 1) * P
                        ]

                        def mm(a, b, start, stop):
                            _lab("mm", f"sc.j{j}.t{tt}.h{h}")
                            nc.tensor.matmul(
                                sc[:, h, a:b],
                                lhsT,
                                qt_sbs[j][64 * h : 64 * h + 64, a:b],
                                start=start,
                                stop=stop,
                            )

                    if k < 0:
                        mm(0, CH, True, True)
                    else:
                        # one group per bank: the first pass zeroes the whole
                        # 2KB zero-region, later passes accumulate
                        if o + P < CH:
                            mm(o + P, CH, True, False)
                            mm(o, o + P, False, False)
                        else:
                            mm(o, o + P, True, False)
                        # diagonal 128-block [o:o+P] gets the ramp mask added
                        _lab("mm", f"mask.j{j}.t{tt}.h{h}")
                        nc.tensor.matmul(
                            sc[:, h, o : o + P],
                            maskA_sb[:],
                            maskB_sb[:],
                            start=False,
                            stop=True,
                        )
                return sc, o

            def emit_exp(j, tt, sc, o):
                pt = ptp.tile([P, HPC, CH], bf16, tag="pt", name="pt")
                nc.scalar.activation(pt[:, :, o:], sc[:, :, o:], EXP, scale=0.125)
                return pt

            def emit_av(j, tt, pt):
                k = tt - 4 * j
                vt = v_sbs[tt // 4]
                for i in range(max(k, 0), 4):
                    for h in range(HPC):
                        _lab("mm", f"av.j{j}.t{tt}.i{i}.h{h}")
                        nc.tensor.matmul(
                            avs[i // 2][:, h, i % 2, 0:65],
                            pt[:, h, i * P : (i + 1) * P],
                            vt[:, tt % 4, h, :],
                            # one accumulation group per bank (zero region):
                            # first pass zeroes it, last pass closes it
                            start=(tt == 0 and i % 2 == 0 and h == 0),
                            stop=(tt == 4 * j + i and i % 2 == 1 and h == 1),
                        )

            def emit_div(j, i, otT, tail=False):
                av = avs[i // 2]
                rc = rcp.tile([P, HPC, 1], f32, tag="rc", name="rc")
                nc.vector.reciprocal(rc[:], av[:, :, i % 2, 64:65])
                ot = otp.tile([P, P], f32, tag="ot", name="ot")
                for h in range(HPC):
                    if False:
                        nc.scalar.mul(
                            ot[:, HD * h : HD * (h + 1)],
                            av[:, h, i % 2, 0:64],
                            rc[:, h, :],
                        )
                    else:
                        nc.vector.tensor_scalar(
                            out=ot[:, HD * h : HD * (h + 1)],
                            in0=av[:, h, i % 2, 0:64],
                            scalar1=rc[:, h, :],
                            scalar2=None,
                            op0=MULT,
                        )
                # PE-transpose (f32) into av's dead region, then copy out
                # to SBUF as bf16; avoids the serialized HWDGE path
                tp = av[:, 0, i % 2, 0:P]
                _lab("mm", f"tp.j{j}.i{i}")
                nc.tensor.transpose(tp, ot[:], ident_sb[:])
                nc.vector.tensor_copy(otT[:, i, :], tp)

            def make_outproj_pieces(j, otT):
                yt_sb = ytp.tile([P, ND, CH], bf16, tag="yt", name="yt")
                rhs = otT[:].rearrange("p i f -> p (i f)")

                def piece(e):
                    def f():
                        ps = ytpp.tile([P, CH], f32, tag="ytps", name="ytps")
                        _lab("mm", f"op.j{j}.e{e}")
                        nc.tensor.matmul(
                            ps[:], wo_sb[:, e, :], rhs, start=True, stop=True
                        )
                        nc.vector.tensor_copy(yt_sb[:, e, :], ps[:])
                        if e == ND - 1:
                            nc.sync.dma_start(
                                yt_d[:, j * CH : (j + 1) * CH].rearrange(
                                    "(e p) s -> p e s", p=P
                                ),
                                yt_sb[:],
                            )

                    return f

                return [piece(e) for e in range(ND)]

            op_q = []
            otTs = {}

            def emit_div_pair(jj, pair, otT, tail=False):
                for i in (2 * pair, 2 * pair + 1):
                    emit_div(jj, i, otT, tail=tail)

            tail_state = {}

            def emit_tail_half(jj, half, otT):
                a, b = half * 256, half * 256 + 256
                if half == 0:
                    tail_state["yt"] = ytp.tile(
                        [P, ND, CH], bf16, tag="yt", name="yt"
                    )
                yt_sb = tail_state["yt"]
                rhs = otT[:, 2 * half : 2 * half + 2, :].rearrange(
                    "p i f -> p (i f)"
                )
                for e in range(ND):
                    if e % 2 == 0:
                        ps = ytpp.tile([P, CH], f32, tag="ytps", name="ytt")
                    else:
                        ps = qkvp.tile([P, CH], f32, tag="qv", name="ytt")
                    _lab("mm", f"op.j{jj}.e{e}.h{half}")
                    nc.tensor.matmul(
                        ps[:, 0:256], wo_sb[:, e, :], rhs, start=True, stop=True
                    )
                    if e % 2 == 0:
                        nc.scalar.copy(yt_sb[:, e, a:b], ps[:, 0:256])
                    else:
                        nc.vector.tensor_copy(yt_sb[:, e, a:b], ps[:, 0:256])
                nc.sync.dma_start(
                    yt_d[:, jj * CH + a : jj * CH + b].rearrange(
                        "(e p) s -> p e s", p=P
                    ),
                    yt_sb[:, :, a:b],
                )

            def handle_pop(j):
                jj, t0, p0_ = fifo.pop(0)
                emit_av(jj, t0, p0_)
                k0 = t0 - 4 * jj
                tail = jj == NCHUNK - 1
                if k0 == 1:
                    otTs[jj] = otTp.tile([P, 4, P], bf16, tag="otT", name="otT")
                    emit_div_pair(jj, 0, otTs[jj], tail=tail)
                    if tail:
                        emit_tail_half(jj, 0, otTs[jj])
                elif k0 == 3:
                    emit_div_pair(jj, 1, otTs[jj], tail=tail)
                    if tail:
                        emit_tail_half(jj, 1, otTs.pop(jj))
                    else:
                        op_q.extend(make_outproj_pieces(jj, otTs.pop(jj)))

            # ---- merged software-pipelined stream ----
            p0 = make_qkv_pieces(0)
            nc.sync.dma_start(
                w_first_sb[:, :, P : 2 * P],
                w_first_d[:, P : 2 * P].rearrange("(dt p) f -> p dt f", p=P),
            )
            nc.sync.dma_start(
                b_first_sb[:], b_first_d[:].rearrange("(c p) -> p c", p=P)
            )
            nc.sync.dma_start(
                wv_sb[:], wv_d[:].rearrange("(dt p) f -> p dt f", p=P)
            )
            p0[0]()
            p0[1]()
            # masks needed by the first diagonal exp (~10us in)
            nc.sync.dma_start(maskA_sb[:], maskA_d[:])
            nc.sync.dma_start(maskB_sb[:], maskB_d[:])
            nc.sync.dma_start(ident_sb[:], ident_d[:])
            # xt(1) prefetch must beat the cold-path weight loads below
            pieces = {1: p0[2:] + make_qkv_pieces(1)}
            if USE_FP8:
                nc.sync.dma_start(
                    wqk_sb[:], wqk_d[:].rearrange("(dt p) f -> p dt f", p=P)
                )
                nc.sync.dma_start(
                    bqk_sb[:], bqk_d[:].rearrange("(c p) -> p c", p=P)
                )
            nc.sync.dma_start(
                wo_sb[:], wo_d[:].rearrange("p (e f) -> p e f", f=P)
            )
            fifo = []
            for j in range(NCHUNK):
                ntt = 4 * j + 4
                cur = pieces.pop(j + 1, [])
                for tt in range(ntt):
                    sc, o = emit_scores(j, tt)
                    pt = emit_exp(j, tt, sc, o)
                    fifo.append((j, tt, pt))
                    if tt >= 1 and cur:
                        cur.pop(0)()
                    if len(fifo) > (4 if j == 0 else 2):
                        handle_pop(j)
                    if tt == max(3, ntt // 2) and j + 2 < NCHUNK:
                        pieces[j + 2] = make_qkv_pieces(j + 2)
                    if tt >= 6 and op_q:
                        op_q.pop(0)()
                while cur:
                    cur.pop(0)()
            while fifo:
                handle_pop(NCHUNK - 1)
            while op_q:
                op_q.pop(0)()

    return nc


@functools.lru_cache(maxsize=1)
def _get_nc(S_arg=S):
    nc = build_nc()
    nc.compile()
    return nc


def make_in_maps(input, Wqkv, bqkv, Wo):
    x = np.asarray(input, dtype=np.float32).reshape(S, D)
    xt = np.ascontiguousarray(x.T).astype(BF16)
    Wqkv = np.asarray(Wqkv, dtype=np.float32)
    bqkv = np.asarray(bqkv, dtype=np.float32)
    Wo = np.asarray(Wo, dtype=np.float32)
    Wq, Wk, Wv = Wqkv[:, 0:D], Wqkv[:, D : 2 * D], Wqkv[:, 2 * D : 3 * D]
    bq, bk = bqkv[0:D], bqkv[D : 2 * D]

    r = np.arange(P)
    maskA = np.where(r[:, None] < r[None, :], np.float32(-1e30), np.float32(0))
    maskB = (r[:, None] >= r[None, :]).astype(np.float32)
    maskA = np.ascontiguousarray(maskA.astype(BF16))
    maskB = np.ascontiguousarray(maskB.astype(BF16))
    ident = np.ascontiguousarray(np.eye(P, dtype=np.float32))

    # fp8 DoubleRow wants [h0 d0-31 | h1 d0-31 | h0 d32-63 | h1 d32-63]
    perm = np.r_[0:32, 64:96, 32:64, 96:128] if USE_FP8 else np.arange(128)

    in_maps = []
    for c in range(NCORES):
        hs = [HPC * c + i for i in range(HPC)]

        def headcols(W):
            return np.concatenate([W[:, h * HD : (h + 1) * HD] for h in hs], 1)

        def headvec(b):
            return np.concatenate([b[h * HD : (h + 1) * HD] for h in hs], 0)

        wq0, wk0 = headcols(Wq), headcols(Wk)
        bq0, bk0 = headvec(bq), headvec(bk)
        wq, wk = wq0[:, perm], wk0[:, perm]
        bq_l, bk_l = bq0[perm], bk0[perm]
        in_maps.append(
            {
                "xt": xt,
                "wqk": np.ascontiguousarray(
                    np.concatenate([wq, wk], axis=1).astype(BF16)
                ),
                "wqk0": np.ascontiguousarray(
                    np.concatenate([wq0, wk0], axis=1).astype(BF16)
                ),
                "bqk0": np.ascontiguousarray(
                    np.concatenate([bq0, bk0], axis=0).astype(np.float32)
                ),
                "wv": np.ascontiguousarray(headcols(Wv).astype(BF16)),
                "wo": np.ascontiguousarray(
                    Wo[hs[0] * HD : hs[0] * HD + HPC * HD, :].astype(BF16)
                ),
                "bqk": np.ascontiguousarray(
                    np.concatenate([bq_l, bk_l], axis=0).astype(np.float32)
                ),
                "maskA": maskA,
                "maskB": maskB,
                "ident": ident,
            }
        )
    return in_maps


def kernel(input, Wqkv, bqkv, Wo, bo):
    from concourse.bass_utils import run_bass_kernel_spmd

    nc = _get_nc()
    in_maps = make_in_maps(input, Wqkv, bqkv, Wo)
    res = None
    last_exc = None
    for _attempt in range(3):  # transient NRT/device errors: retry
        try:
            res = run_bass_kernel_spmd(nc, in_maps, core_ids=list(range(NCORES)))
            break
        except Exception as e:  # noqa: BLE001
            last_exc = e
    if res is None:
        raise last_exc
    acc = np.zeros((D, S), np.float32)
    for r in res.results:
        acc += np.asarray(r["yt"], dtype=np.float32)
    y = np.ascontiguousarray(acc.T)
    bv = np.asarray(bqkv, np.float32)[2 * D : 3 * D]
    y += (bv @ np.asarray(Wo, np.float32) + np.asarray(bo, np.float32))[None, :]
    return y.reshape(1, S, D)
